# revision 1
# baseline (speedup 1.0000x reference)
"""Multi-head attention (B=4, S=2048, D=768, H=12) on 8 Trainium2 cores.

Sharding: the 48 (batch, head) pairs are data-parallel; each core gets 6.
Per head on one core (all matmuls bf16, fp32 PSUM accumulation):
  QT/KT [128, S]  : col/row-duplicated so score matmuls can row-tile
                    (two K=64 matmuls run concurrently in the PE array)
  V     [S, 64+1] : ones column appended -> AV matmul also produces the
                    softmax denominator (normalization folded to the end)
  scoresT [k, q]  : per 128-row k-chunk, [128, 1024] PSUM tiles
  P = exp(s/8)    : ACT engine, PSUM -> SBUF bf16 (the throughput wall)
  out^T [65, 512] = sum_k V_aug^T P; row 64 = softmax denominator.
                    Denominator rows bounce through DRAM and return as one
                    [128, 16] tile for a single cheap reciprocal; a 0-stride
                    DMA re-read broadcasts each reciprocal row across
                    partitions for one DVE mul per 512-chunk. Output lands
                    in [e, q] layout; the host gather transposes it back.

Scheduling: ACT is the bottleneck engine, so score-pair emission (the
only producer of ACT work) is interleaved 1:N with "filler" PE work
(AV matmuls of the previous q-block, QKV of the next head) via a FIFO
of generators, keeping both engines dense.
"""

import sys
from collections import deque

for _p in ("/opt/trn_rl_repo",):
    if _p not in sys.path:
        sys.path.insert(0, _p)

import numpy as np

B, S, D, H = 4, 2048, 768, 12
DH = 64
NCORES = 8
HPC = (B * H) // NCORES  # 6 heads per core
SCALE = 1.0 / 8.0
NKC = S // 128  # 16 k-chunks
NQB = 2  # q blocks of 1024
QB = S // NQB
PUMPS_PER_PAIR = 5


def _split_multi_waits(nc):
    """This walrus build rejects >1 sync wait per instruction. Insert
    single-wait NoOps (same engine, so same instruction stream) ahead of
    any instruction carrying several waits."""
    import bass_rust
    import concourse.mybir as mybir

    n_split = 0
    for f in nc.m.functions:
        for bb in f.blocks:
            out = []
            dirty = False
            for inst in bb.instructions:
                si = inst.sync_info
                if si is not None and len(si.on_wait) > 1:
                    waits = list(si.on_wait)
                    for j, w in enumerate(waits[:-1]):
                        nop = mybir.InstNoOp(name=f"{inst.name}-w{j}", ins=[], outs=[])
                        nop.engine = inst.engine
                        nop.sync_info = bass_rust.SyncInfo(on_wait=[w], on_update=[])
                        out.append(nop)
                    si.on_wait = waits[-1:]
                    dirty = True
                    n_split += 1
                out.append(inst)
            if dirty:
                bb.instructions = out
    return n_split


_BUILT = None


def build():
    global _BUILT
    if _BUILT is not None:
        return _BUILT
    import concourse.bass as bass
    import concourse.mybir as mybir
    import concourse.tile as tile

    F32 = mybir.dt.float32
    BF = mybir.dt.bfloat16
    AF = mybir.ActivationFunctionType

    nc = bass.Bass()
    xTd = nc.dram_tensor("xT", [HPC, 128, S], BF, kind="ExternalInput")
    wqkd = nc.dram_tensor("wqk", [HPC, 64, 2, 64], BF, kind="ExternalInput")
    wvTd = nc.dram_tensor("wvT", [HPC, 128, 64], BF, kind="ExternalInput")
    bqkd = nc.dram_tensor("bqk", [HPC, 128, 2], F32, kind="ExternalInput")
    bvd = nc.dram_tensor("bv", [HPC, 1, 64], F32, kind="ExternalInput")
    outd = nc.dram_tensor("out", [HPC, 64, S], F32, kind="ExternalOutput")
    dnd = nc.dram_tensor("dnd", [HPC, 4, 512], F32)  # denominator bounce
    rcd = nc.dram_tensor("rcd", [HPC, 4, 512], F32)  # reciprocal bounce

    with tile.TileContext(nc) as tc:
        with (
            tc.tile_pool(name="const", bufs=1) as cpool,
            tc.tile_pool(name="x", bufs=2) as xpool,
            tc.tile_pool(name="w", bufs=2) as wpool,
            tc.tile_pool(name="qk", bufs=2) as qkpool,
            tc.tile_pool(name="v", bufs=2) as vpool,
            tc.tile_pool(name="pt", bufs=2 * NKC * NQB) as ptpool,
            tc.tile_pool(name="ot", bufs=9) as otpool,
            tc.tile_pool(name="r", bufs=3) as rpool,
            tc.tile_pool(name="sp", bufs=3, space="PSUM") as sppool,
            tc.tile_pool(name="avp", bufs=2, space="PSUM") as avpool,
        ):
            ones1 = cpool.tile([1, 128], F32, tag="ones1")
            nc.vector.memset(ones1[:], 1.0)

            state = {}

            def qkv_steps(i):
                x_t = xpool.tile([128, S], BF, tag="x", name=f"x{i}")
                nc.gpsimd.dma_start(x_t[:], xTd[i])
                w_t = wpool.tile([64, 2, 64], BF, tag="wqk", name=f"wqk{i}")
                nc.gpsimd.dma_start(w_t[:], wqkd[i])
                wv_t = wpool.tile([128, 64], BF, tag="wv", name=f"wv{i}")
                nc.gpsimd.dma_start(wv_t[:], wvTd[i])
                b_t = wpool.tile([128, 2], F32, tag="bqk", name=f"bqk{i}")
                nc.gpsimd.dma_start(b_t[:], bqkd[i])
                bv_t = wpool.tile([1, 64], F32, tag="bv", name=f"bv{i}")
                nc.gpsimd.dma_start(bv_t[:], bvd[i])

                # bias(V) broadcast along partitions via K=1 outer product
                bv_ps = sppool.tile([128, 64], F32, tag="sp", name=f"bvp{i}")
                nc.tensor.matmul(bv_ps[:], ones1[:], bv_t[:])
                bv_sb = wpool.tile([128, 64], F32, tag="bvsb", name=f"bvs{i}")
                nc.vector.tensor_copy(bv_sb[:], bv_ps[:])
                yield

                # QT/KT duplicated into both partition halves (col-tiled pair)
                qt = qkpool.tile([128, S], BF, tag="qt", name=f"qt{i}")
                kt = qkpool.tile([128, S], BF, tag="kt", name=f"kt{i}")
                for dst, wsel, bsel in ((qt, 0, 0), (kt, 1, 1)):
                    for qm in range(4):
                        ps = sppool.tile(
                            [128, 512], F32, tag="sp", name=f"qk{i}_{wsel}_{qm}"
                        )
                        rhs = x_t[0:64, qm * 512 : (qm + 1) * 512]
                        nc.tensor.matmul(
                            ps[0:64, :], w_t[:, wsel, :], rhs, tile_position=(0, 0)
                        )
                        nc.tensor.matmul(
                            ps[64:128, :], w_t[:, wsel, :], rhs, tile_position=(0, 64)
                        )
                        nc.vector.tensor_scalar_add(
                            dst[:, qm * 512 : (qm + 1) * 512],
                            ps[:],
                            b_t[:, bsel : bsel + 1],
                        )
                        yield

                # V with ones column (denominator trick), row-tiled pairs
                v_sb = vpool.tile([128, NKC, 65], BF, tag="v", name=f"v{i}")
                nc.vector.memset(v_sb[:, :, 64:65], 1.0)
                for p in range(8):
                    psA = sppool.tile([128, 64], F32, tag="sp", name=f"vA{i}_{p}")
                    psB = sppool.tile([128, 64], F32, tag="sp", name=f"vB{i}_{p}")
                    nc.tensor.matmul(
                        psA[:],
                        x_t[0:64, p * 128 : (p + 1) * 128],
                        wv_t[0:64, :],
                        tile_position=(0, 0),
                    )
                    nc.tensor.matmul(
                        psB[:],
                        x_t[64:128, (p + 8) * 128 : (p + 9) * 128],
                        wv_t[64:128, :],
                        tile_position=(64, 0),
                    )
                    nc.vector.tensor_add(v_sb[:, p, 0:64], psA[:], bv_sb[:])
                    nc.vector.tensor_add(v_sb[:, p + 8, 0:64], psB[:], bv_sb[:])
                    if p % 2 == 1:
                        yield
                state[i] = {"qt": qt, "kt": kt, "v": v_sb, "pt": {}}

            def sc_pair(i, jb, kc):
                """One kc-pair of row-tiled score matmuls + their exps."""
                qt, kt = state[i]["qt"], state[i]["kt"]
                pt = state[i]["pt"].setdefault(jb, [None] * NKC)
                tA = sppool.tile([128, QB], F32, tag="sp", name=f"sA{i}_{jb}_{kc}")
                tB = sppool.tile([128, QB], F32, tag="sp", name=f"sB{i}_{jb}_{kc}")
                lA = kt[0:64, kc * 128 : (kc + 1) * 128]
                lB = kt[64:128, (kc + 8) * 128 : (kc + 9) * 128]
                # same-lhsT matmuls adjacent so ldw-opt can skip the reload
                for qm in range(QB // 512):
                    q0 = jb * QB + qm * 512
                    sl = slice(qm * 512, (qm + 1) * 512)
                    nc.tensor.matmul(
                        tA[:, sl], lA, qt[0:64, q0 : q0 + 512], tile_position=(0, 0)
                    )
                for qm in range(QB // 512):
                    q0 = jb * QB + qm * 512
                    sl = slice(qm * 512, (qm + 1) * 512)
                    nc.tensor.matmul(
                        tB[:, sl], lB, qt[64:128, q0 : q0 + 512], tile_position=(64, 0)
                    )
                pA = ptpool.tile([128, QB], BF, tag="pt", name=f"pA{i}_{jb}_{kc}")
                pB = ptpool.tile([128, QB], BF, tag="pt", name=f"pB{i}_{jb}_{kc}")
                nc.scalar.activation(pA[:], tA[:], AF.Exp, scale=SCALE)
                nc.scalar.activation(pB[:], tB[:], AF.Exp, scale=SCALE)
                pt[kc] = pA
                pt[kc + 8] = pB

            def av_steps(i, jb):
                """Generator: AV matmuls in groups of 4; numerators parked in
                SBUF, denominator rows bounced to DRAM. After the last
                q-chunk of the head: one [128,16] reciprocal, bounce back,
                broadcast-read per chunk, multiply, store."""
                v_sb = state[i]["v"]
                pt = state[i]["pt"].pop(jb)
                ots_list = state[i].setdefault("ots", [])
                for qm in range(QB // 512):
                    av = avpool.tile([128, 512], F32, tag="av", name=f"av{i}_{jb}_{qm}")
                    for kc in range(NKC):
                        nc.tensor.matmul(
                            av[0:65, :],
                            v_sb[:, kc, :],
                            pt[kc][:, qm * 512 : (qm + 1) * 512],
                            start=(kc == 0),
                            stop=(kc == NKC - 1),
                        )
                        if kc % 4 == 3:
                            yield
                    g = jb * (QB // 512) + qm
                    ots = otpool.tile([65, 512], F32, tag="ot", name=f"ot{i}_{jb}_{qm}")
                    nc.vector.tensor_copy(ots[:], av[0:65, :])
                    nc.sync.dma_start(dnd[i, g : g + 1], ots[64:65, :])
                    ots_list.append(ots)
                    yield
                if jb == NQB - 1:
                    den4 = rpool.tile([128, 16], F32, tag="r", name=f"dn{i}")
                    nc.sync.dma_start(
                        den4[:], dnd[i].rearrange("a c -> (a c)").rearrange("(p c) -> p c", c=16)
                    )
                    r4 = rpool.tile([128, 16], F32, tag="r", name=f"rc{i}")
                    nc.vector.reciprocal(r4[:], den4[:])
                    nc.sync.dma_start(
                        rcd[i].rearrange("a c -> (a c)").rearrange("(p c) -> p c", c=16),
                        r4[:],
                    )
                    yield
                    for g, ots in enumerate(ots_list):
                        rb = rpool.tile([64, 512], F32, tag="rb", name=f"rb{i}_{g}")
                        nc.sync.dma_start(
                            rb[:],
                            rcd[i, g]
                            .rearrange("(a n) -> a n", a=1)
                            .to_broadcast((64, 512)),
                        )
                        nc.vector.tensor_mul(ots[0:64, :], ots[0:64, :], rb[:])
                        nc.sync.dma_start(
                            outd[i][:, g * 512 : (g + 1) * 512], ots[0:64, :]
                        )
                        yield
                    state[i]["ots"] = []

            fillers = deque()

            def pump(n):
                while n > 0 and fillers:
                    try:
                        next(fillers[0])
                        n -= 1
                    except StopIteration:
                        fillers.popleft()

            def drain(gen=None):
                while fillers and (gen is None or gen in fillers):
                    pump(1)

            def unit(i, jb):
                for kc in range(NKC // 2):
                    sc_pair(i, jb, kc)
                    pump(PUMPS_PER_PAIR)

            # head 0 QKV runs eagerly; afterwards QKV(i+1) + AV trail the
            # score stream as interleaved filler, lagging by one q-block
            g0 = qkv_steps(0)
            fillers.append(g0)
            drain(g0)
            unit(0, 0)
            for i in range(HPC):
                if i > 0:
                    fillers.append(av_steps(i - 1, 1))
                    unit(i, 0)
                fillers.append(av_steps(i, 0))
                if i + 1 < HPC:
                    g = qkv_steps(i + 1)
                    fillers.append(g)
                    unit(i, 1)
                    drain(g)
                else:
                    unit(i, 1)
            fillers.append(av_steps(HPC - 1, 1))
            drain()

    _split_multi_waits(nc)
    _BUILT = nc
    return nc


def _core_inputs(sequences, wq, bq, wk, bk, wv, bv):
    import ml_dtypes

    bf16 = ml_dtypes.bfloat16
    xh = np.asarray(sequences, dtype=np.float32).reshape(B, S, H, DH)
    wq, bq = np.asarray(wq, np.float32), np.asarray(bq, np.float32)
    wk, bk = np.asarray(wk, np.float32), np.asarray(bk, np.float32)
    wv, bv = np.asarray(wv, np.float32), np.asarray(bv, np.float32)
    in_maps = []
    for c in range(NCORES):
        xT = np.empty((HPC, 128, S), dtype=bf16)
        wqk = np.empty((HPC, 64, 2, 64), dtype=bf16)
        wvT = np.empty((HPC, 128, 64), dtype=bf16)
        bqk = np.empty((HPC, 128, 2), dtype=np.float32)
        bvv = np.empty((HPC, 1, 64), dtype=np.float32)
        for i in range(HPC):
            f = c * HPC + i
            b, h = f // H, f % H
            xt = np.ascontiguousarray(xh[b, :, h, :].T).astype(bf16)
            xT[i, 0:64] = xt
            xT[i, 64:128] = xt
            wqk[i, :, 0, :] = wq[h].T.astype(bf16)
            wqk[i, :, 1, :] = wk[h].T.astype(bf16)
            wvT[i, 0:64] = wv[h].T.astype(bf16)
            wvT[i, 64:128] = wv[h].T.astype(bf16)
            bqk[i, 0:64, 0] = bq[h]
            bqk[i, 64:128, 0] = bq[h]
            bqk[i, 0:64, 1] = bk[h]
            bqk[i, 64:128, 1] = bk[h]
            bvv[i, 0] = bv[h]
        in_maps.append({"xT": xT, "wqk": wqk, "wvT": wvT, "bqk": bqk, "bv": bvv})
    return in_maps


def _gather(results):
    out = np.empty((B, S, H, DH), np.float32)
    for c in range(NCORES):
        o = np.asarray(results[c]["out"])  # [HPC, 64, S]
        for i in range(HPC):
            f = c * HPC + i
            b, h = f // H, f % H
            out[b, :, h, :] = o[i].T
    return out.reshape(B, S, D)


def kernel(sequences, wq, bq, wk, bk, wv, bv):
    from concourse.bass_utils import run_bass_kernel_spmd

    nc = build()
    in_maps = _core_inputs(sequences, wq, bq, wk, bk, wv, bv)
    res = run_bass_kernel_spmd(nc, in_maps, list(range(NCORES)))
    return _gather(res.results)



# revision 3
# speedup vs baseline: 1.0950x; 1.0950x over previous
"""Multi-head attention (B=4, S=2048, D=768, H=12) on 8 Trainium2 cores.

Sharding: the 48 (batch, head) pairs are data-parallel; each core gets 6.
Per head on one core (all matmuls bf16, fp32 PSUM accumulation):
  x_aug [65, S]   : x^T with a ones row appended; QKV biases fold into the
                    projection matmuls (K=65 contraction), so PSUM->SBUF
                    moves are pure copies.
  QT/KT [128, S]  : q/k duplicated into both partition halves via col-tiled
                    projection pairs; enables row-tiled score pairs.
  scoresT [k, q]  : per kc-pair, tA/tB [128, 1024] PSUM tiles produced by
                    interleaved A/B matmuls on disjoint PE row halves
                    (tile_position (0,0)/(64,0)) so the HW overlaps them.
  P = exp(s/8)    : split across TWO engines: tA -> ACT (true exp, bf16),
                    tB -> DVE bit-trick exp (one tensor_scalar computing
                    round(s*16*log2e + (16256-C)) into uint16 = the bf16
                    bit pattern of 2^(s/(8 ln2)); negative saturation gives
                    +0.0, i.e. clean underflow). Halves the exp wall.
  V     [S, 64+1] : ones column appended -> AV matmul also produces the
                    softmax denominator.
  AV out [65, 512]: V_aug^T P accumulated over 16 k-chunks per 512-q chunk.
  norm            : out^T chunks are PE-transposed ([65,128] -> [128,65]
                    PSUM) so the denominator lands as a per-partition
                    column; one DVE reciprocal + per-partition tensor_scalar
                    multiply normalizes. No DRAM bounce, no serialized tail.
  out [S, 64] fp32 per head, gathered host-side with no transpose.

Scheduling: score-pair emission is interleaved with filler PE work (AV of
the previous block, QKV of the next head) via a FIFO of generators, keeping
PE dense while ACT/DVE drain the exp stream.
"""

import sys
from collections import deque

for _p in ("/opt/trn_rl_repo",):
    if _p not in sys.path:
        sys.path.insert(0, _p)

import numpy as np

B, S, D, H = 4, 2048, 768, 12
DH = 64
NCORES = 8
HPC = (B * H) // NCORES  # 6 heads per core
SCALE = 1.0 / 8.0
NKC = S // 128  # 16 k-chunks
NQB = 2  # q blocks of 1024
QB = S // NQB
PUMPS_PER_PAIR = 3
# DVE bit-trick exp constants: bits = round(s * 16*log2e + (16256 - C))
EXP_C = 5.8
DVE_A = float(16.0 / np.log(2.0))
DVE_B = float(127.0 * 128.0 - EXP_C)


def _split_multi_waits(nc):
    """This walrus build rejects >1 sync wait per instruction. Insert
    single-wait NoOps (same engine, so same instruction stream) ahead of
    any instruction carrying several waits."""
    import bass_rust
    import concourse.mybir as mybir

    n_split = 0
    for f in nc.m.functions:
        for bb in f.blocks:
            out = []
            dirty = False
            for inst in bb.instructions:
                si = inst.sync_info
                if si is not None and len(si.on_wait) > 1:
                    waits = list(si.on_wait)
                    for j, w in enumerate(waits[:-1]):
                        nop = mybir.InstNoOp(name=f"{inst.name}-w{j}", ins=[], outs=[])
                        nop.engine = inst.engine
                        nop.sync_info = bass_rust.SyncInfo(on_wait=[w], on_update=[])
                        out.append(nop)
                    si.on_wait = waits[-1:]
                    dirty = True
                    n_split += 1
                out.append(inst)
            if dirty:
                bb.instructions = out
    return n_split


_BUILT = None


def build():
    global _BUILT
    if _BUILT is not None:
        return _BUILT
    import concourse.bass as bass
    import concourse.mybir as mybir
    import concourse.tile as tile

    F32 = mybir.dt.float32
    BF = mybir.dt.bfloat16
    U16 = mybir.dt.uint16
    AF = mybir.ActivationFunctionType
    ALU = mybir.AluOpType

    nc = bass.Bass()
    xTd = nc.dram_tensor("xT", [HPC, 65, S], BF, kind="ExternalInput")
    wqkd = nc.dram_tensor("wqk", [HPC, 65, 2, 64], BF, kind="ExternalInput")
    wvTd = nc.dram_tensor("wvT", [HPC, 65, 64], BF, kind="ExternalInput")
    identd = nc.dram_tensor("ident", [65, 65], F32, kind="ExternalInput")
    outd = nc.dram_tensor("out", [HPC, S, 64], F32, kind="ExternalOutput")

    with tile.TileContext(nc) as tc:
        with (
            tc.tile_pool(name="const", bufs=1) as cpool,
            tc.tile_pool(name="x", bufs=2) as xpool,
            tc.tile_pool(name="w", bufs=2) as wpool,
            tc.tile_pool(name="qk", bufs=2) as qkpool,
            tc.tile_pool(name="v", bufs=2) as vpool,
            tc.tile_pool(name="pt", bufs=2 * NKC * NQB) as ptpool,
            tc.tile_pool(name="ot", bufs=6) as otpool,
            tc.tile_pool(name="r", bufs=4) as rpool,
            tc.tile_pool(name="ob", bufs=6) as opool,
            tc.tile_pool(name="sp", bufs=2, space="PSUM") as sppool,
            tc.tile_pool(name="qv", bufs=2, space="PSUM") as qvpool,
            tc.tile_pool(name="avp", bufs=2, space="PSUM") as avpool,
        ):
            ident = cpool.tile([65, 65], F32, tag="id")
            nc.sync.dma_start(ident[:], identd[:])

            state = {}

            def qkv_steps(i):
                x_t = xpool.tile([65, S], BF, tag="x", name=f"x{i}")
                nc.gpsimd.dma_start(x_t[:], xTd[i])
                w_t = wpool.tile([65, 2, 64], BF, tag="wqk", name=f"wqk{i}")
                nc.gpsimd.dma_start(w_t[:], wqkd[i])
                wv_t = wpool.tile([65, 64], BF, tag="wv", name=f"wv{i}")
                nc.gpsimd.dma_start(wv_t[:], wvTd[i])
                yield

                # QT/KT duplicated into both partition halves (col-tiled
                # concurrent pair); bias folded via the ones row (K=65).
                qt = qkpool.tile([128, S], BF, tag="qt", name=f"qt{i}")
                kt = qkpool.tile([128, S], BF, tag="kt", name=f"kt{i}")
                for dst, wsel in ((qt, 0), (kt, 1)):
                    for qm in range(4):
                        ps = qvpool.tile(
                            [128, 512], F32, tag="qv", name=f"qk{i}_{wsel}_{qm}"
                        )
                        rhs = x_t[0:65, qm * 512 : (qm + 1) * 512]
                        nc.tensor.matmul(
                            ps[0:64, :], w_t[:, wsel, :], rhs, tile_position=(0, 0)
                        )
                        nc.tensor.matmul(
                            ps[64:128, :], w_t[:, wsel, :], rhs, tile_position=(0, 64)
                        )
                        nc.scalar.copy(dst[:, qm * 512 : (qm + 1) * 512], ps[:])
                        yield

                # V with ones column (denominator trick); bias folded.
                v_sb = vpool.tile([128, NKC, 65], BF, tag="v", name=f"v{i}")
                nc.vector.memset(v_sb[:, :, 64:65], 1.0)
                for p in range(8):
                    ps_v = qvpool.tile([128, 2, 64], F32, tag="qv", name=f"vp{i}_{p}")
                    nc.tensor.matmul(
                        ps_v[:, 0, :],
                        x_t[0:65, (2 * p) * 128 : (2 * p + 1) * 128],
                        wv_t[:],
                    )
                    nc.tensor.matmul(
                        ps_v[:, 1, :],
                        x_t[0:65, (2 * p + 1) * 128 : (2 * p + 2) * 128],
                        wv_t[:],
                    )
                    nc.vector.tensor_copy(v_sb[:, 2 * p : 2 * p + 2, 0:64], ps_v[:])
                    if p % 2 == 1:
                        yield
                state[i] = {"qt": qt, "kt": kt, "v": v_sb, "pt": {}}

            def sc_pair(i, jb, kc):
                """One kc-pair of row-tiled score matmuls, A/B interleaved so
                the PE overlaps them; exp split across ACT (tA) / DVE (tB)."""
                qt, kt = state[i]["qt"], state[i]["kt"]
                pt = state[i]["pt"].setdefault(jb, [None] * NKC)
                tA = sppool.tile([128, QB], F32, tag="sp", name=f"sA{i}_{jb}_{kc}")
                tB = sppool.tile([128, QB], F32, tag="sp", name=f"sB{i}_{jb}_{kc}")
                lA = kt[0:64, kc * 128 : (kc + 1) * 128]
                lB = kt[64:128, (kc + 8) * 128 : (kc + 9) * 128]
                for qm in range(QB // 512):
                    q0 = jb * QB + qm * 512
                    sl = slice(qm * 512, (qm + 1) * 512)
                    nc.tensor.matmul(
                        tA[:, sl], lA, qt[0:64, q0 : q0 + 512], tile_position=(0, 0)
                    )
                    nc.tensor.matmul(
                        tB[:, sl], lB, qt[64:128, q0 : q0 + 512], tile_position=(64, 0)
                    )
                pA = ptpool.tile([128, QB], BF, tag="pt", name=f"pA{i}_{jb}_{kc}")
                nc.scalar.activation(pA[:], tA[:], AF.Exp, scale=SCALE)
                pB = ptpool.tile([128, QB], U16, tag="pt", name=f"pB{i}_{jb}_{kc}")
                nc.vector.tensor_scalar(pB[:], tB[:], DVE_A, DVE_B, ALU.mult, ALU.add)
                pt[kc] = (pA, False)
                pt[kc + 8] = (pB, True)

            def av_steps(i, jb):
                """Generator: AV matmuls per 512-q chunk; then PE-transpose
                the [65, 512] result so the denominator becomes a
                per-partition column; reciprocal + per-partition multiply
                normalizes; store [128, 64] fp32 tiles."""
                v_sb = state[i]["v"]
                pt = state[i]["pt"].pop(jb)
                ots_list = []
                for g in range(QB // 512):
                    av = avpool.tile([65, 512], F32, tag="av", name=f"av{i}_{jb}_{g}")
                    for kc in range(NKC):
                        t, is_u16 = pt[kc]
                        rhs = t[:, g * 512 : (g + 1) * 512]
                        if is_u16:
                            rhs = rhs.bitcast(mybir.dt.bfloat16)
                        nc.tensor.matmul(
                            av[:],
                            v_sb[:, kc, :],
                            rhs,
                            start=(kc == 0),
                            stop=(kc == NKC - 1),
                        )
                        if kc % 4 == 3:
                            yield
                    ots = otpool.tile([65, 512], F32, tag="ot", name=f"ot{i}_{jb}_{g}")
                    nc.scalar.copy(ots[:], av[:])
                    ots_list.append(ots)
                    yield
                for g, ots in enumerate(ots_list):
                    tp = avpool.tile([128, 4, 65], F32, tag="av", name=f"tp{i}_{jb}_{g}")
                    for c in range(4):
                        nc.tensor.transpose(
                            tp[:, c, :], ots[0:65, c * 128 : (c + 1) * 128], ident[:]
                        )
                    r = rpool.tile([128, 4, 1], F32, tag="r", name=f"r{i}_{jb}_{g}")
                    nc.vector.reciprocal(r[:], tp[:, :, 64:65])
                    yield
                    q0 = jb * QB + g * 512
                    for c in range(4):
                        osb = opool.tile([128, 64], F32, tag="ob", name=f"o{i}_{jb}_{g}_{c}")
                        nc.vector.tensor_scalar_mul(osb[:], tp[:, c, 0:64], r[:, c, :])
                        nc.sync.dma_start(
                            outd[i, q0 + c * 128 : q0 + (c + 1) * 128, :], osb[:]
                        )
                    yield

            fillers = deque()

            def pump(n):
                while n > 0 and fillers:
                    try:
                        next(fillers[0])
                        n -= 1
                    except StopIteration:
                        fillers.popleft()

            def drain(gen=None):
                while fillers and (gen is None or gen in fillers):
                    pump(1)

            def unit(i, jb):
                for kc in range(NKC // 2):
                    sc_pair(i, jb, kc)
                    pump(PUMPS_PER_PAIR)

            # head 0 QKV runs eagerly; afterwards QKV(i+1) + AV trail the
            # score stream as interleaved filler, lagging by one q-block
            g0 = qkv_steps(0)
            fillers.append(g0)
            drain(g0)
            unit(0, 0)
            for i in range(HPC):
                if i > 0:
                    fillers.append(av_steps(i - 1, 1))
                    unit(i, 0)
                fillers.append(av_steps(i, 0))
                if i + 1 < HPC:
                    g = qkv_steps(i + 1)
                    fillers.append(g)
                    unit(i, 1)
                    drain(g)
                else:
                    unit(i, 1)
            fillers.append(av_steps(HPC - 1, 1))
            drain()

    _split_multi_waits(nc)
    _BUILT = nc
    return nc


def _core_inputs(sequences, wq, bq, wk, bk, wv, bv):
    import ml_dtypes

    bf16 = ml_dtypes.bfloat16
    xh = np.asarray(sequences, dtype=np.float32).reshape(B, S, H, DH)
    wq, bq = np.asarray(wq, np.float32), np.asarray(bq, np.float32)
    wk, bk = np.asarray(wk, np.float32), np.asarray(bk, np.float32)
    wv, bv = np.asarray(wv, np.float32), np.asarray(bv, np.float32)
    ident = np.eye(65, dtype=np.float32)
    in_maps = []
    for c in range(NCORES):
        xT = np.empty((HPC, 65, S), dtype=bf16)
        wqk = np.empty((HPC, 65, 2, 64), dtype=bf16)
        wvT = np.empty((HPC, 65, 64), dtype=bf16)
        for i in range(HPC):
            f = c * HPC + i
            b, h = f // H, f % H
            xT[i, 0:64] = np.ascontiguousarray(xh[b, :, h, :].T).astype(bf16)
            xT[i, 64] = np.ones(S, dtype=bf16)
            wqk[i, 0:64, 0, :] = wq[h].T.astype(bf16)
            wqk[i, 0:64, 1, :] = wk[h].T.astype(bf16)
            wqk[i, 64, 0, :] = bq[h].astype(bf16)
            wqk[i, 64, 1, :] = bk[h].astype(bf16)
            wvT[i, 0:64] = wv[h].T.astype(bf16)
            wvT[i, 64] = bv[h].astype(bf16)
        in_maps.append({"xT": xT, "wqk": wqk, "wvT": wvT, "ident": ident})
    return in_maps


def _gather(results):
    out = np.empty((B, S, H, DH), np.float32)
    for c in range(NCORES):
        o = np.asarray(results[c]["out"])  # [HPC, S, 64]
        for i in range(HPC):
            f = c * HPC + i
            b, h = f // H, f % H
            out[b, :, h, :] = o[i]
    return out.reshape(B, S, D)


def kernel(sequences, wq, bq, wk, bk, wv, bv):
    from concourse.bass_utils import run_bass_kernel_spmd

    nc = build()
    in_maps = _core_inputs(sequences, wq, bq, wk, bk, wv, bv)
    res = run_bass_kernel_spmd(nc, in_maps, list(range(NCORES)))
    return _gather(res.results)


# revision 5
# speedup vs baseline: 1.1393x; 1.0405x over previous
"""Multi-head attention (B=4, S=2048, D=768, H=12) on 8 Trainium2 cores.

Sharding: the 48 (batch, head) pairs are data-parallel; each core gets 6.
Per head on one core (all matmuls bf16, fp32 PSUM accumulation):
  x_aug [65, S]   : x^T with a ones row appended; QKV biases fold into the
                    projection matmuls (K=65 contraction), so PSUM->SBUF
                    moves are pure copies.
  QT/KT [128, S]  : q/k duplicated into both partition halves via col-tiled
                    projection pairs; enables row-tiled score pairs.
  scoresT [k, q]  : per kc-pair, tA/tB [128, 1024] PSUM tiles produced by
                    interleaved A/B matmuls on disjoint PE row halves
                    (tile_position (0,0)/(64,0)) so the HW overlaps them.
  P = exp(s/8)    : split across TWO engines: tA -> ACT (true exp, bf16),
                    tB -> DVE bit-trick exp (one tensor_scalar computing
                    round(s*16*log2e + (16256-C)) into uint16 = the bf16
                    bit pattern of 2^(s/(8 ln2)); negative saturation gives
                    +0.0, i.e. clean underflow). Halves the exp wall.
  V     [S, 64+1] : ones column appended -> AV matmul also produces the
                    softmax denominator.
  AV out [65, 512]: V_aug^T P accumulated over 16 k-chunks per 512-q chunk.
  norm            : out^T chunks are PE-transposed ([65,128] -> [128,65]
                    PSUM) so the denominator lands as a per-partition
                    column; one DVE reciprocal + per-partition tensor_scalar
                    multiply normalizes. No DRAM bounce, no serialized tail.
  out [S, 64] fp32 per head, gathered host-side with no transpose.

Scheduling: score-pair emission is interleaved with filler PE work (AV of
the previous block, QKV of the next head) via a FIFO of generators, keeping
PE dense while ACT/DVE drain the exp stream.
"""

import sys
from collections import deque

for _p in ("/opt/trn_rl_repo",):
    if _p not in sys.path:
        sys.path.insert(0, _p)

import numpy as np

B, S, D, H = 4, 2048, 768, 12
DH = 64
NCORES = 8
HPC = (B * H) // NCORES  # 6 heads per core
SCALE = 1.0 / 8.0
NKC = S // 128  # 16 k-chunks
NQB = 2  # q blocks of 1024
QB = S // NQB
PUMPS_PER_PAIR = 4
# DVE bit-trick exp constants: bits = round(s * 16*log2e + (16256 - C))
EXP_C = 5.8
DVE_A = float(16.0 / np.log(2.0))
DVE_B = float(127.0 * 128.0 - EXP_C)


def _split_multi_waits(nc):
    """This walrus build rejects >1 sync wait per instruction. Insert
    single-wait NoOps (same engine, so same instruction stream) ahead of
    any instruction carrying several waits."""
    import bass_rust
    import concourse.mybir as mybir

    n_split = 0
    for f in nc.m.functions:
        for bb in f.blocks:
            out = []
            dirty = False
            for inst in bb.instructions:
                si = inst.sync_info
                if si is not None and len(si.on_wait) > 1:
                    waits = list(si.on_wait)
                    for j, w in enumerate(waits[:-1]):
                        nop = mybir.InstNoOp(name=f"{inst.name}-w{j}", ins=[], outs=[])
                        nop.engine = inst.engine
                        nop.sync_info = bass_rust.SyncInfo(on_wait=[w], on_update=[])
                        out.append(nop)
                    si.on_wait = waits[-1:]
                    dirty = True
                    n_split += 1
                out.append(inst)
            if dirty:
                bb.instructions = out
    return n_split


_BUILT = None


def build():
    global _BUILT
    if _BUILT is not None:
        return _BUILT
    import concourse.bass as bass
    import concourse.mybir as mybir
    import concourse.tile as tile

    F32 = mybir.dt.float32
    BF = mybir.dt.bfloat16
    U16 = mybir.dt.uint16
    AF = mybir.ActivationFunctionType
    ALU = mybir.AluOpType

    nc = bass.Bass()
    xTd = nc.dram_tensor("xT", [HPC, 65, S], BF, kind="ExternalInput")
    wqkd = nc.dram_tensor("wqk", [HPC, 65, 2, 64], BF, kind="ExternalInput")
    wvTd = nc.dram_tensor("wvT", [HPC, 65, 64], BF, kind="ExternalInput")
    identd = nc.dram_tensor("ident", [65, 65], F32, kind="ExternalInput")
    outd = nc.dram_tensor("out", [HPC, S, 64], F32, kind="ExternalOutput")

    with tile.TileContext(nc) as tc:
        with (
            tc.tile_pool(name="const", bufs=1) as cpool,
            tc.tile_pool(name="x", bufs=2) as xpool,
            tc.tile_pool(name="w", bufs=2) as wpool,
            tc.tile_pool(name="qk", bufs=2) as qkpool,
            tc.tile_pool(name="v", bufs=2) as vpool,
            tc.tile_pool(name="pt", bufs=2 * NKC * NQB) as ptpool,
            tc.tile_pool(name="ot", bufs=6) as otpool,
            tc.tile_pool(name="r", bufs=4) as rpool,
            tc.tile_pool(name="ob", bufs=6) as opool,
            tc.tile_pool(name="sp", bufs=3, space="PSUM") as sppool,
            tc.tile_pool(name="avp", bufs=2, space="PSUM") as avpool,
        ):
            ident = cpool.tile([65, 65], F32, tag="id")
            nc.sync.dma_start(ident[:], identd[:])

            state = {}

            def qkv_steps(i):
                x_t = xpool.tile([65, S], BF, tag="x", name=f"x{i}")
                nc.gpsimd.dma_start(x_t[:], xTd[i])
                w_t = wpool.tile([65, 2, 64], BF, tag="wqk", name=f"wqk{i}")
                nc.gpsimd.dma_start(w_t[:], wqkd[i])
                wv_t = wpool.tile([65, 64], BF, tag="wv", name=f"wv{i}")
                nc.gpsimd.dma_start(wv_t[:], wvTd[i])
                yield

                # QT/KT duplicated into both partition halves (col-tiled
                # concurrent pair); bias folded via the ones row (K=65).
                qt = qkpool.tile([128, S], BF, tag="qt", name=f"qt{i}")
                kt = qkpool.tile([128, S], BF, tag="kt", name=f"kt{i}")
                for dst, wsel in ((qt, 0), (kt, 1)):
                    for qm in range(4):
                        ps = sppool.tile(
                            [128, 512], F32, tag="sp", name=f"qk{i}_{wsel}_{qm}"
                        )
                        rhs = x_t[0:65, qm * 512 : (qm + 1) * 512]
                        nc.tensor.matmul(
                            ps[0:64, :], w_t[:, wsel, :], rhs, tile_position=(0, 0)
                        )
                        nc.tensor.matmul(
                            ps[64:128, :], w_t[:, wsel, :], rhs, tile_position=(0, 64)
                        )
                        nc.scalar.copy(dst[:, qm * 512 : (qm + 1) * 512], ps[:])
                        yield

                # V with ones column (denominator trick); bias folded.
                v_sb = vpool.tile([128, NKC, 65], BF, tag="v", name=f"v{i}")
                nc.vector.memset(v_sb[:, :, 64:65], 1.0)
                for p in range(8):
                    ps_v = sppool.tile([128, 2, 64], F32, tag="sp", name=f"vp{i}_{p}")
                    nc.tensor.matmul(
                        ps_v[:, 0, :],
                        x_t[0:65, (2 * p) * 128 : (2 * p + 1) * 128],
                        wv_t[:],
                    )
                    nc.tensor.matmul(
                        ps_v[:, 1, :],
                        x_t[0:65, (2 * p + 1) * 128 : (2 * p + 2) * 128],
                        wv_t[:],
                    )
                    nc.vector.tensor_copy(v_sb[:, 2 * p : 2 * p + 2, 0:64], ps_v[:])
                    if p % 2 == 1:
                        yield
                state[i] = {"qt": qt, "kt": kt, "v": v_sb, "pt": {}}

            def sc_pair(i, jb, kc):
                """One kc-pair of row-tiled score matmuls, A/B interleaved so
                the PE overlaps them; exp split across ACT (tA) / DVE (tB)."""
                qt, kt = state[i]["qt"], state[i]["kt"]
                pt = state[i]["pt"].setdefault(jb, [None] * NKC)
                tA = sppool.tile([128, QB], F32, tag="sp", name=f"sA{i}_{jb}_{kc}")
                tB = sppool.tile([128, QB], F32, tag="sp", name=f"sB{i}_{jb}_{kc}")
                lA = kt[0:64, kc * 128 : (kc + 1) * 128]
                lB = kt[64:128, (kc + 8) * 128 : (kc + 9) * 128]
                for qm in range(QB // 512):
                    q0 = jb * QB + qm * 512
                    sl = slice(qm * 512, (qm + 1) * 512)
                    nc.tensor.matmul(
                        tA[:, sl], lA, qt[0:64, q0 : q0 + 512], tile_position=(0, 0)
                    )
                    nc.tensor.matmul(
                        tB[:, sl], lB, qt[64:128, q0 : q0 + 512], tile_position=(64, 0)
                    )
                pA = ptpool.tile([128, QB], BF, tag="pt", name=f"pA{i}_{jb}_{kc}")
                nc.scalar.activation(pA[:], tA[:], AF.Exp, scale=SCALE)
                pB = ptpool.tile([128, QB], U16, tag="pt", name=f"pB{i}_{jb}_{kc}")
                nc.vector.tensor_scalar(pB[:], tB[:], DVE_A, DVE_B, ALU.mult, ALU.add)
                pt[kc] = (pA, False)
                pt[kc + 8] = (pB, True)

            def av_steps(i, jb):
                """Generator: AV matmuls per 512-q chunk; then PE-transpose
                the [65, 512] result so the denominator becomes a
                per-partition column; reciprocal + per-partition multiply
                normalizes; store [128, 64] fp32 tiles."""
                v_sb = state[i]["v"]
                pt = state[i]["pt"].pop(jb)
                ots_list = []
                for g in range(QB // 512):
                    av = avpool.tile([65, 512], F32, tag="av", name=f"av{i}_{jb}_{g}")
                    for kc in range(NKC):
                        t, is_u16 = pt[kc]
                        rhs = t[:, g * 512 : (g + 1) * 512]
                        if is_u16:
                            rhs = rhs.bitcast(mybir.dt.bfloat16)
                        nc.tensor.matmul(
                            av[:],
                            v_sb[:, kc, :],
                            rhs,
                            start=(kc == 0),
                            stop=(kc == NKC - 1),
                        )
                        if kc % 2 == 1:
                            yield
                    ots = otpool.tile([65, 512], F32, tag="ot", name=f"ot{i}_{jb}_{g}")
                    nc.scalar.copy(ots[:], av[:])
                    ots_list.append(ots)
                    yield
                for g, ots in enumerate(ots_list):
                    tp = avpool.tile([128, 4, 65], F32, tag="av", name=f"tp{i}_{jb}_{g}")
                    for c in range(4):
                        nc.tensor.transpose(
                            tp[:, c, :], ots[0:65, c * 128 : (c + 1) * 128], ident[:]
                        )
                    r = rpool.tile([128, 4, 1], F32, tag="r", name=f"r{i}_{jb}_{g}")
                    nc.vector.reciprocal(r[:], tp[:, :, 64:65])
                    yield
                    q0 = jb * QB + g * 512
                    osb = opool.tile([128, 4, 64], F32, tag="ob", name=f"o{i}_{jb}_{g}")
                    for c in range(4):
                        nc.vector.tensor_scalar_mul(
                            osb[:, c, :], tp[:, c, 0:64], r[:, c, :]
                        )
                    nc.sync.dma_start(
                        outd[i, q0 : q0 + 512, :].rearrange(
                            "(c p) e -> p c e", c=4
                        ),
                        osb[:],
                    )
                    yield

            fillers = deque()

            def pump(n):
                while n > 0 and fillers:
                    try:
                        next(fillers[0])
                        n -= 1
                    except StopIteration:
                        fillers.popleft()

            def drain(gen=None):
                while fillers and (gen is None or gen in fillers):
                    pump(1)

            def unit(i, jb):
                for kc in range(NKC // 2):
                    sc_pair(i, jb, kc)
                    pump(PUMPS_PER_PAIR)

            # head 0 QKV runs eagerly; afterwards QKV(i+1) + AV trail the
            # score stream as interleaved filler, lagging by one q-block
            g0 = qkv_steps(0)
            fillers.append(g0)
            drain(g0)
            qg = qkv_steps(1)
            fillers.append(qg)
            unit(0, 0)
            for i in range(HPC):
                if i > 0:
                    fillers.append(av_steps(i - 1, 1))
                    unit(i, 0)
                fillers.append(av_steps(i, 0))
                unit(i, 1)
                if qg is not None:
                    drain(qg)
                qg = qkv_steps(i + 2) if i + 2 < HPC else None
                if qg is not None:
                    fillers.append(qg)
            fillers.append(av_steps(HPC - 1, 1))
            drain()

    _split_multi_waits(nc)
    _BUILT = nc
    return nc


def _core_inputs(sequences, wq, bq, wk, bk, wv, bv):
    import ml_dtypes

    bf16 = ml_dtypes.bfloat16
    xh = np.asarray(sequences, dtype=np.float32).reshape(B, S, H, DH)
    wq, bq = np.asarray(wq, np.float32), np.asarray(bq, np.float32)
    wk, bk = np.asarray(wk, np.float32), np.asarray(bk, np.float32)
    wv, bv = np.asarray(wv, np.float32), np.asarray(bv, np.float32)
    ident = np.eye(65, dtype=np.float32)
    in_maps = []
    for c in range(NCORES):
        xT = np.empty((HPC, 65, S), dtype=bf16)
        wqk = np.empty((HPC, 65, 2, 64), dtype=bf16)
        wvT = np.empty((HPC, 65, 64), dtype=bf16)
        for i in range(HPC):
            f = c * HPC + i
            b, h = f // H, f % H
            xT[i, 0:64] = np.ascontiguousarray(xh[b, :, h, :].T).astype(bf16)
            xT[i, 64] = np.ones(S, dtype=bf16)
            wqk[i, 0:64, 0, :] = wq[h].T.astype(bf16)
            wqk[i, 0:64, 1, :] = wk[h].T.astype(bf16)
            wqk[i, 64, 0, :] = bq[h].astype(bf16)
            wqk[i, 64, 1, :] = bk[h].astype(bf16)
            wvT[i, 0:64] = wv[h].T.astype(bf16)
            wvT[i, 64] = bv[h].astype(bf16)
        in_maps.append({"xT": xT, "wqk": wqk, "wvT": wvT, "ident": ident})
    return in_maps


def _gather(results):
    out = np.empty((B, S, H, DH), np.float32)
    for c in range(NCORES):
        o = np.asarray(results[c]["out"])  # [HPC, S, 64]
        for i in range(HPC):
            f = c * HPC + i
            b, h = f // H, f % H
            out[b, :, h, :] = o[i]
    return out.reshape(B, S, D)


def kernel(sequences, wq, bq, wk, bk, wv, bv):
    from concourse.bass_utils import run_bass_kernel_spmd

    nc = build()
    in_maps = _core_inputs(sequences, wq, bq, wk, bk, wv, bv)
    res = run_bass_kernel_spmd(nc, in_maps, list(range(NCORES)))
    return _gather(res.results)
